# revision 33
# baseline (speedup 1.0000x reference)
"""Trainium2 Bass kernel for a DoReFa-quantized ResNet BasicBlock.

    out = act(bn2(conv3x3(act(bn1(conv3x3(x, qw(w1)))), qw(w2))) + x)

with 4-bit DoReFa weight/activation quantization and training-mode BatchNorm
(batch statistics over N,H,W).

Strategy (8 NeuronCores, data-parallel over batch):
 - batch N=64 sharded 8 images/core; weights replicated.
 - BN uses *synced* batch statistics: per-core per-channel mean/var come from
   one-pass vector bn_stats/bn_aggr, converted to sum/sumsq and AllGathered
   across the 8 cores (two tiny [128,2] collectives).
 - conv3x3 = 9 shifted matmuls accumulated in PSUM (C_in on partitions,
   pixels on free dim), using a zero-padded [C,58,58] SBUF image.
 - Weight quantization produces small odd integers (2m-15, |.| <= 15) which
   are exact in bf16/fp8; the /15 scales are folded into the BN affine maps.
   conv1 runs single-pass bf16 (x rounded once to bf16, ~2^-9 relative);
   conv2 inputs are exact small ints and runs in fp8 exactly, with DoubleRow
   perf mode fusing tap pairs (two 3x3 taps per matmul).
 - Activation quantization uses the 2^23 magic-constant round-to-nearest-even
   (matches jnp.round) after clipping via min/max; the final residual path is
   computed on the x15 scale so relu folds into the rounding pass.
"""

import numpy as np

import bass_rust
import concourse.bacc as bacc
import concourse.mybir as mybir
import concourse.tile as tile
import concourse.bass_isa as bass_isa
from concourse.bass_utils import run_bass_kernel_spmd
from concourse.bass_interp import get_hw_module
from concourse.masks import make_identity

F32 = mybir.dt.float32
BF16 = mybir.dt.bfloat16
F16 = mybir.dt.float16
FP8 = mybir.dt.float8e4
AF = mybir.ActivationFunctionType
ALU = mybir.AluOpType
DR = mybir.MatmulPerfMode.DoubleRow

N_CORES = 8
N_PER = 8            # images per core
C = 128              # channels
H = W = 56
HW = H * W           # 3136
QHW = HW // 4        # quarter image pixels
PW = 58              # padded height/width
RCH = 8              # output rows per chunk
NCHUNK = H // RCH    # 7 chunks per image
CHN = RCH * W        # 448 pixels per chunk
NREC = N_PER * NCHUNK
MAGIC = float(2.0 ** 23)
N_SHARD = float(N_PER * HW)           # per-core BN sample count
N_BATCH = 64 * HW                     # full-batch BN sample count
INV_N = float(np.float32(1.0 / N_BATCH))
EPS1 = float(np.float32(225e-5))      # 15^2 * 1e-5   (conv1 output scale)
EPS2 = float(np.float32(50625e-5))    # 225^2 * 1e-5  (conv2 output scale)
INV15 = float(np.float32(1.0 / 15.0))
# tanh(w) ~ w * (1 + w2*(c1 + w2*(c2 + w2*c3))), |w| < 0.25
TC1 = float(np.float32(-1.0 / 3.0))
TC2 = float(np.float32(2.0 / 15.0))
TC3 = float(np.float32(-17.0 / 315.0))
TAPS = [(ky, kx) for ky in range(3) for kx in range(3)]
TAP_OFF = [ky * PW + kx for ky, kx in TAPS]

_CACHED = {}


def _pair_rhs(apad_ap, r0, p):
    """Overlapping AP selecting the two shifted conv windows of tap pair p:
    [C, 2, RCH, W] where dim1 steps between tap offsets (DoubleRow rhs)."""
    ky0, kx0 = TAPS[2 * p]
    delta = TAP_OFF[2 * p + 1] - TAP_OFF[2 * p]
    base = apad_ap[:, r0 + ky0:r0 + ky0 + RCH, kx0:kx0 + W]
    u = base.unsqueeze(1).broadcast_to((C, 2, RCH, W)).copy()
    pairs = [tuple(x) for x in u.ap]
    pairs[1] = (delta, 2)
    u.ap = bass_rust.VecI64Pair(pairs)
    return u


def _border_zero(nc, pad_ap):
    """Zero just the 1-px border ring of a [C, PW, PW] padded tile (the
    interior is fully overwritten) - 4 small DVE memsets instead of one
    slow full-tile gpsimd memset."""
    nc.vector.memset(pad_ap[:, 0:1, :], 0.0)
    nc.vector.memset(pad_ap[:, PW - 1:PW, :], 0.0)
    nc.vector.memset(pad_ap[:, 1:PW - 1, 0:1], 0.0)
    nc.vector.memset(pad_ap[:, 1:PW - 1, PW - 1:PW], 0.0)


def _tanh_poly_multi(nc, parts):
    """wt = taylor_tanh(w) elementwise over several (out, tmp, w2, w) slice
    groups, ops interleaved across groups to hide DVE inter-op latency."""
    steps = [
        lambda o, t, w2, w: nc.vector.tensor_tensor(w2, w, w, ALU.mult),
        lambda o, t, w2, w: nc.vector.tensor_scalar(t, w2, TC3, TC2,
                                                    ALU.mult, ALU.add),
        lambda o, t, w2, w: nc.vector.tensor_tensor(t, t, w2, ALU.mult),
        lambda o, t, w2, w: nc.vector.tensor_scalar(t, t, TC1, None, ALU.add),
        lambda o, t, w2, w: nc.vector.tensor_tensor(t, t, w2, ALU.mult),
        lambda o, t, w2, w: nc.vector.tensor_tensor(o, w, t, ALU.mult),
        lambda o, t, w2, w: nc.vector.tensor_tensor(o, w, o, ALU.add),
    ]
    for step in steps:
        for grp in parts:
            step(*grp)


def _tanh_poly(nc, tt_out, ts_tmp, w2src, wsrc):
    _tanh_poly_multi(nc, [(tt_out, ts_tmp, w2src, wsrc)])


def _quant_stats(nc, consts, wsb, name):
    """Global max |w| across all partitions (the only gpsimd step of the
    weight quant - hoisted early so it is not stuck behind the warm-up
    collective trigger, which blocks the gpsimd queue until the NEFF init
    barrier completes)."""
    amax = consts.tile([C, 1], F32, tag=f"amax{name}")
    nc.vector.tensor_reduce(amax[:], wsb, mybir.AxisListType.X, ALU.max,
                            apply_absolute_value=True)
    gmax = consts.tile([C, 1], F32, tag=f"gmax{name}")
    nc.gpsimd.partition_all_reduce(gmax[:], amax[:], C, bass_isa.ReduceOp.max)
    return gmax


def _quant_weights_both(nc, pool_T, consts, ptr, ident, wsb, gmaxes,
                        copy_fns):
    """DoReFa-quantize BOTH [128,128,3,3] weights (DMA'd side by side into
    wsb [C, 2*C*9]) in one fused chain - the two halves double as the
    latency-hiding interleave groups.  Per-tap transposed integer (2m-15)
    tiles are written via copy_fns[k](tap_index, psum_bf16_ap)."""
    K = C * 9
    halves = [slice(0, K), slice(K, 2 * K)]
    # tanh(max) per weight (tanh is monotone; same f32 poly as below)
    s15s = []
    parts = []
    for k in (0, 1):
        mt1 = consts.tile([C, 1], F32, tag=f"mt1{k}")
        mt2 = consts.tile([C, 1], F32, tag=f"mt2{k}")
        mval = consts.tile([C, 1], F32, tag=f"mval{k}")
        parts.append((mval[:], mt1[:], mt2[:], gmaxes[k][:]))
    _tanh_poly_multi(nc, parts)
    for k in (0, 1):
        mval = parts[k][0]
        inv2m = consts.tile([C, 1], F32, tag=f"inv2m{k}")
        nc.vector.tensor_scalar(inv2m[:], mval, 2.0, None, ALU.mult)
        nc.vector.reciprocal(inv2m[:], inv2m[:])
        s15 = consts.tile([C, 1], F32, tag=f"s15{k}")
        nc.vector.tensor_scalar(s15[:], inv2m[:], 15.0, None, ALU.mult)
        s15s.append(s15)
    w2t = pool_T.tile([C, 2 * K], F32, tag="T")
    qt = pool_T.tile([C, 2 * K], F32, tag="T")
    wt = pool_T.tile([C, 2 * K], F32, tag="T")
    wn = pool_T.tile([C, 2 * K], F32, tag="T")
    wi = pool_T.tile([C, 2 * K], BF16, tag="T")
    _tanh_poly_multi(nc, [(wt[:, hs], qt[:, hs], w2t[:, hs], wsb[:, hs])
                          for hs in halves])
    for k, hs in enumerate(halves):
        nc.vector.tensor_scalar(wn[:, hs], wt[:, hs], s15s[k][:, 0:1], 7.5,
                                ALU.mult, ALU.add)
    for hs in halves:
        nc.vector.tensor_scalar(wn[:, hs], wn[:, hs], MAGIC, -MAGIC,
                                ALU.add, ALU.add)
    for hs in halves:
        nc.vector.tensor_scalar(wi[:, hs], wn[:, hs], 2.0, -15.0,
                                ALU.mult, ALU.add)
    # transpose each tap via PE: lhsT[i, o] = Wi[o, i*9+t]
    for k in (0, 1):
        wir = wi[:, halves[k]].rearrange("o (i t) -> o i t", t=9)
        for t in range(9):
            pst = ptr.tile([C, C], BF16, tag="tr")
            nc.tensor.transpose(pst[:], wir[:, :, t], ident[:])
            copy_fns[k](t, pst)


def _stats_to_sums(nc, statsp, stats_rec, nsamp, name):
    """bn_aggr [C, k, 6] records into [C,2] (mean,var), convert to
    [sum, sumsq] over the nsamp samples they cover."""
    mv = statsp.tile([C, 2], F32, tag=f"mv{name}")
    nc.vector.bn_aggr(mv[:], stats_rec)
    msq = statsp.tile([C, 1], F32, tag=f"msq{name}")
    nc.vector.tensor_tensor(msq[:], mv[:, 0:1], mv[:, 0:1], ALU.mult)
    st = statsp.tile([C, 2], F32, tag=f"st{name}")
    nc.vector.tensor_scalar(st[:, 0:1], mv[:, 0:1], nsamp, None, ALU.mult)
    nc.vector.tensor_tensor(msq[:], mv[:, 1:2], msq[:], ALU.add)
    nc.vector.tensor_scalar(st[:, 1:2], msq[:], nsamp, None, ALU.mult)
    return st


def _ag_sum(nc, statsp, dram, st, RG, name):
    """Cross-core sum of a [C,2] stats tile via AllGather + local reduce
    (AG has a lower latency floor than AllReduce for tiny payloads).
    The staging DMAs ride the sync queue - its ring is kept warm by the
    image loads/prefetches, while a cold ring adds ~6us latency."""
    agi = dram.tile([C, 2], F32, tag=f"agi{name}")
    ago = dram.tile([N_CORES, C, 2], F32, tag=f"ago{name}")
    nc.sync.dma_start(agi[:], st[:])
    nc.gpsimd.collective_compute(
        "AllGather", ALU.bypass, replica_groups=RG,
        ins=[agi.opt()], outs=[ago.opt()])
    allst = statsp.tile([C, 2, N_CORES], F32, tag=f"allst{name}")
    nc.sync.dma_start(allst[:], ago.rearrange("r c s -> c s r"))
    rst = statsp.tile([C, 2], F32, tag=f"rst{name}")
    nc.vector.tensor_reduce(rst[:], allst[:], mybir.AxisListType.X, ALU.add)
    return rst


def _load_gb(nc, consts, gamma_ap, beta_ap, name):
    """Preload gamma/beta on the scalar DMA queue (done at build start so
    they are never stuck behind the phase-3 x prefetch on the sync queue)."""
    g = consts.tile([C, 1], F32, tag=f"g{name}")
    nc.scalar.dma_start(g[:], gamma_ap.rearrange("(c one) -> c one", one=1))
    b = consts.tile([C, 1], F32, tag=f"b{name}")
    nc.scalar.dma_start(b[:], beta_ap.rearrange("(c one) -> c one", one=1))
    return g, b


def _bn_vectors(nc, consts, rstats, g, b, eps, post_scale, name):
    """Build per-channel scale/bias [128,1] s.t. T*scale + bias equals
    post_scale * batchnorm(T/k); eps is pre-scaled by k^2."""
    mean = consts.tile([C, 1], F32, tag=f"mean{name}")
    nc.vector.tensor_scalar(mean[:], rstats[:, 0:1], INV_N, None, ALU.mult)
    var = consts.tile([C, 1], F32, tag=f"var{name}")
    nc.vector.tensor_scalar(var[:], rstats[:, 1:2], INV_N, None, ALU.mult)
    msq = consts.tile([C, 1], F32, tag=f"msq{name}")
    nc.vector.tensor_tensor(msq[:], mean[:], mean[:], ALU.mult)
    nc.vector.tensor_tensor(var[:], var[:], msq[:], ALU.subtract)
    epst = consts.tile([C, 1], F32, tag=f"eps{name}")
    nc.vector.memset(epst[:], eps)
    std = consts.tile([C, 1], F32, tag=f"std{name}")
    nc.scalar.activation(std[:], var[:], AF.Sqrt, bias=epst[:, 0:1], scale=1.0)
    inv = consts.tile([C, 1], F32, tag=f"inv{name}")
    nc.vector.reciprocal(inv[:], std[:])
    scale = consts.tile([C, 1], F32, tag=f"scale{name}")
    nc.vector.tensor_tensor(scale[:], g[:], inv[:], ALU.mult)
    nc.vector.tensor_scalar(scale[:], scale[:], post_scale, None, ALU.mult)
    bias = consts.tile([C, 1], F32, tag=f"bias{name}")
    nc.vector.tensor_tensor(bias[:], mean[:], scale[:], ALU.mult)
    nc.vector.tensor_scalar(b[:], b[:], post_scale, None, ALU.mult)
    nc.vector.tensor_tensor(bias[:], b[:], bias[:], ALU.subtract)
    return scale, bias


def build():
    nc = bacc.Bacc("TRN2", target_bir_lowering=False, debug=False,
                   num_devices=N_CORES)
    x_ap = nc.dram_tensor("x", [N_PER, C, H, W], F32, kind="ExternalInput").ap()
    w1_ap = nc.dram_tensor("w1", [C, C, 3, 3], F32, kind="ExternalInput").ap()
    w2_ap = nc.dram_tensor("w2", [C, C, 3, 3], F32, kind="ExternalInput").ap()
    g1_ap = nc.dram_tensor("gamma1", [C], F32, kind="ExternalInput").ap()
    b1_ap = nc.dram_tensor("beta1", [C], F32, kind="ExternalInput").ap()
    g2_ap = nc.dram_tensor("gamma2", [C], F32, kind="ExternalInput").ap()
    b2_ap = nc.dram_tensor("beta2", [C], F32, kind="ExternalInput").ap()
    out_ap = nc.dram_tensor("out", [N_PER, C, H, W], F32,
                            kind="ExternalOutput").ap()
    x_r = x_ap.rearrange("n c h w -> n c h w")
    x_f = x_ap.rearrange("n c h w -> n c (h w)")
    out_f = out_ap.rearrange("n c h w -> n c (h w)")
    RG = [list(range(N_CORES))]

    with tile.TileContext(nc) as tc:
        with tc.tile_pool(name="consts", bufs=1) as consts, \
             tc.tile_pool(name="T", bufs=N_PER) as pool_T, \
             tc.tile_pool(name="padhl", bufs=4) as padhl, \
             tc.tile_pool(name="apad", bufs=3) as apadp, \
             tc.tile_pool(name="xio", bufs=2) as xio, \
             tc.tile_pool(name="ximg", bufs=10) as ximgp, \
             tc.tile_pool(name="yimg", bufs=6) as yimgp, \
             tc.tile_pool(name="psum", bufs=5, space="PSUM") as psum, \
             tc.tile_pool(name="ptr", bufs=2, space="PSUM") as ptr, \
             tc.tile_pool(name="stats", bufs=1) as statsp, \
             tc.tile_pool(name="dram", bufs=1, space="DRAM") as dram:

            # weight DMAs issue first on the scalar queue so the wquant
            # chain is never stuck behind the 6.4MB of image loads
            wsb = pool_T.tile([C, 2 * C * 9], F32, tag="T")
            nc.scalar.dma_start(wsb[:, 0:C * 9],
                                w1_ap.rearrange("o i kh kw -> o (i kh kw)"))
            nc.scalar.dma_start(wsb[:, C * 9:],
                                w2_ap.rearrange("o i kh kw -> o (i kh kw)"))

            ident = consts.tile([C, C], BF16, tag="ident")
            make_identity(nc, ident[:])
            lhsT1 = consts.tile([C, 9, C], F16, tag="lhsT1")
            # conv2 weights: 4 DoubleRow pairs + 1 single, fp8
            lhsT2p = consts.tile([C, 4, 2, C], FP8, tag="lhsT2p")
            lhsT2s = consts.tile([C, C], FP8, tag="lhsT2s")

            # lhsT copies ride the DVE queue: the ACT queue carries the
            # image-prep copies, which wait on image DMAs - Tile's
            # counter-based sems would make conv1's first matmul wait for
            # ALL earlier ACT work (tens of us of DMA) otherwise.
            def copy1(t, pst):
                nc.vector.tensor_copy(lhsT1[:, t, :], pst[:])

            def copy2(t, pst):
                if t < 8:
                    nc.vector.tensor_copy(lhsT2p[:, t // 2, t % 2, :], pst[:])
                else:
                    nc.vector.tensor_copy(lhsT2s[:], pst[:])

            def prep_image(i):
                xp = padhl.tile([C, PW, PW], F16, tag="pad")
                _border_zero(nc, xp)
                for g in range(4):
                    xs = xio.tile([C, 14, W], F32, tag="xio")
                    nc.sync.dma_start(xs[:], x_r[i, :, g * 14:(g + 1) * 14, :])
                    nc.scalar.copy(xp[:, 1 + g * 14:1 + (g + 1) * 14, 1:57],
                                   xs[:])
                return xp

            g1, b1 = _load_gb(nc, consts, g1_ap, b1_ap, "1")
            g2, b2 = _load_gb(nc, consts, g2_ap, b2_ap, "2")
            prefetched = []

            stats1 = statsp.tile([C, NREC, 6], F32, tag="stats1")
            T1 = []

            def conv1_image(i, xp):
                Ti = pool_T.tile([C, HW], F32, tag="T")
                T1.append(Ti)
                Tir = Ti.rearrange("c (h w) -> c h w", w=W)
                for ck in range(NCHUNK):
                    ps = psum.tile([C, CHN], F32, tag="mm")
                    r0 = ck * RCH
                    for k, (ky, kx) in enumerate(TAPS):
                        rhs = xp[:, r0 + ky:r0 + ky + RCH, kx:kx + W]
                        nc.tensor.matmul(ps[:], lhsT1[:, k, :], rhs,
                                         start=(k == 0), stop=(k == 8))
                    col = i * NCHUNK + ck
                    nc.scalar.copy(Tir[:, r0:r0 + RCH, :], ps[:])
                    nc.vector.bn_stats(stats1[:, col, :], ps[:])
                if i == 6:
                    # mid-flight cross-core sync to absorb skew before AR1
                    ccs = dram.tile([C, 6], F32, tag="ccs")
                    ccso = dram.tile([C, 6], F32, tag="ccso")
                    nc.sync.dma_start(ccs[:], stats1[:, 6 * NCHUNK, :])
                    nc.gpsimd.collective_compute(
                        "AllReduce", ALU.add, replica_groups=RG,
                        ins=[ccs.opt()], outs=[ccso.opt()])

            # prep image 0 first (only DMA/ACT/DVE), then quantize w1 so
            # conv1 can start as soon as lhsT1 is ready.  The warm-up
            # collective trigger goes AFTER wquant1's gpsimd reduce (the
            # trigger blocks the gpsimd queue until the cc stream is up),
            # and wquant2 goes AFTER conv1 image 0 (its PE transposes would
            # otherwise delay conv1's matmuls in the in-order tensor queue).
            xp0 = prep_image(0)
            gmax1 = _quant_stats(nc, consts, wsb[:, 0:C * 9], "1")
            gmax2 = _quant_stats(nc, consts, wsb[:, C * 9:], "2")

            # dummy collective to absorb mesh/barrier warmup during conv1;
            # its input is copied from gmax2 to force the gpsimd queue to
            # run both partition reduces BEFORE the trigger (the trigger
            # stalls the queue until the NEFF init barrier completes)
            ccwi = dram.tile([C, 1], F32, tag="ccwi")
            ccwo = dram.tile([C, 1], F32, tag="ccwo")
            nc.gpsimd.dma_start(ccwi[:], gmax2[:])
            nc.gpsimd.collective_compute(
                "AllReduce", ALU.add, replica_groups=RG,
                ins=[ccwi.opt()], outs=[ccwo.opt()])

            with nc.named_scope("wquant"):
                _quant_weights_both(nc, pool_T, consts, ptr, ident, wsb,
                                    [gmax1, gmax2], [copy1, copy2])
            xp1 = prep_image(1)
            with nc.named_scope("conv1_img0"):
                conv1_image(0, xp0)

            # ---------------- phase 1: conv1 + stats ----------------
            st1a = None
            with nc.named_scope("conv1"):
                for i in range(1, N_PER):
                    xp = xp1 if i == 1 else prep_image(i)
                    conv1_image(i, xp)
                    if i == 6:
                        # aggregate images 0-6 while image 7 is convolving
                        st1a = _stats_to_sums(nc, statsp,
                                              stats1[:, 0:7 * NCHUNK, :],
                                              float(7 * HW), "1a")

            # allreduce stats 1 (image 7's records + the precomputed rest)
            st1b = _stats_to_sums(nc, statsp, stats1[:, 7 * NCHUNK:, :],
                                  float(HW), "1b")
            st1 = statsp.tile([C, 2], F32, tag="st1")
            nc.vector.tensor_tensor(st1[:], st1a[:], st1b[:], ALU.add)
            rst1 = _ag_sum(nc, statsp, dram, st1, RG, "1")
            sc1, bi1 = _bn_vectors(nc, consts, rst1, g1, b1, EPS1, 15.0, "1")

            # ---------------- phase 2: act1 + conv2 + stats ----------------
            stats2 = statsp.tile([C, NREC, 6], F32, tag="stats2")
            T2 = []
            with nc.named_scope("act1_conv2"):
                for i in range(N_PER):
                    ap_t = apadp.tile([C, PW, PW], FP8, tag="apad")
                    _border_zero(nc, ap_t)
                    Tir = T1[i]
                    for q in range(4):
                        y = yimgp.tile([C, 14, W], F32, tag="yimg")
                        nc.scalar.activation(
                            y[:], Tir[:, q * QHW:(q + 1) * QHW].rearrange(
                                "c (h w) -> c h w", w=W),
                            AF.Relu, bias=bi1[:, 0:1], scale=sc1[:, 0:1])
                        nc.vector.tensor_scalar(y[:], y[:], 15.0, MAGIC,
                                                ALU.min, ALU.add)
                        nc.scalar.activation(
                            ap_t[:, 1 + q * 14:1 + (q + 1) * 14, 1:57], y[:],
                            AF.Copy, bias=-MAGIC, scale=1.0)
                    Ti2 = pool_T.tile([C, HW], F32, tag="T")
                    T2.append(Ti2)
                    T2r = Ti2.rearrange("c (h w) -> c h w", w=W)
                    for ck in range(NCHUNK):
                        ps = psum.tile([C, CHN], F32, tag="mm")
                        r0 = ck * RCH
                        for p in range(4):
                            nc.tensor.matmul(ps[:], lhsT2p[:, p, :, :],
                                             _pair_rhs(ap_t, r0, p),
                                             start=(p == 0), stop=False,
                                             perf_mode=DR)
                        rhs8 = ap_t[:, r0 + 2:r0 + 2 + RCH, 2:2 + W]
                        nc.tensor.matmul(ps[:], lhsT2s[:], rhs8,
                                         start=False, stop=True)
                        col = i * NCHUNK + ck
                        nc.any.tensor_copy(T2r[:, r0:r0 + RCH, :], ps[:])
                        nc.vector.bn_stats(stats2[:, col, :], ps[:])
                    # prefetch the residual x quarters for phase 3 on the
                    # (otherwise idle) sync DMA queue - only as many as the
                    # pool holds, so the stats-AG staging DMAs behind them
                    # on the ring are never gated on phase-3 progress
                    while len(prefetched) < 10:
                        j = len(prefetched)
                        pi, pq = divmod(j, 4)
                        xr = ximgp.tile([C, QHW], F32, tag="ximg")
                        nc.sync.dma_start(
                            xr[:], x_f[pi, :, pq * QHW:(pq + 1) * QHW])
                        prefetched.append(xr)
                    if i == 6:
                        st2a = _stats_to_sums(nc, statsp,
                                              stats2[:, 0:7 * NCHUNK, :],
                                              float(7 * HW), "2a")

            # allreduce stats 2 (image 7's records + the precomputed rest)
            st2b = _stats_to_sums(nc, statsp, stats2[:, 7 * NCHUNK:, :],
                                  float(HW), "2b")
            st2 = statsp.tile([C, 2], F32, tag="st2")
            nc.vector.tensor_tensor(st2[:], st2a[:], st2b[:], ALU.add)
            rst2 = _ag_sum(nc, statsp, dram, st2, RG, "2")
            # fold the x15 residual scale into the bn2 affine: y15 = 15*bn2
            sc2, bi2 = _bn_vectors(nc, consts, rst2, g2, b2, EPS2, 15.0, "2")

            # ------------- phase 3: bn2 + residual + act -> out -------------
            # v15 = 15*bn2(T2) + 15*x; codes = relu(rtne(min(v15,15)));
            # out = codes/15
            with nc.named_scope("final"):
                for i in range(N_PER):
                    for q in range(4):
                        sl = slice(q * QHW, (q + 1) * QHW)
                        xr = prefetched[i * 4 + q]
                        # keep the rolling x prefetch ~10 quarters ahead
                        if len(prefetched) < 4 * N_PER:
                            j = len(prefetched)
                            pi, pq = divmod(j, 4)
                            nxr = ximgp.tile([C, QHW], F32, tag="ximg")
                            nc.sync.dma_start(
                                nxr[:], x_f[pi, :, pq * QHW:(pq + 1) * QHW])
                            prefetched.append(nxr)
                        y = yimgp.tile([C, QHW], F32, tag="yimg")
                        nc.scalar.activation(
                            y[:], T2[i][:, sl],
                            AF.Identity, bias=bi2[:, 0:1], scale=sc2[:, 0:1])
                        nc.vector.scalar_tensor_tensor(
                            y[:], xr[:], 15.0, y[:], ALU.mult, ALU.add)
                        nc.vector.tensor_scalar(y[:], y[:], 15.0, MAGIC,
                                                ALU.min, ALU.add)
                        nc.vector.tensor_scalar(y[:], y[:], -MAGIC, 0.0,
                                                ALU.add, ALU.max)
                        # write back into the (now dead) T2 quarter so the
                        # image stores as one contiguous [C,3136] DMA
                        nc.scalar.activation(T2[i][:, sl], y[:], AF.Copy,
                                             bias=0.0, scale=INV15)
                    nc.gpsimd.dma_start(out_f[i, :, :], T2[i][:])

    nc.compile()
    return nc


def kernel(x, w1, w2, gamma1, beta1, gamma2, beta2):
    if "nc" not in _CACHED:
        _CACHED["nc"] = build()
    nc = _CACHED["nc"]
    x = np.ascontiguousarray(x, dtype=np.float32)
    shard = x.reshape(N_CORES, N_PER, C, H, W)
    common = {
        "w1": np.ascontiguousarray(w1, np.float32),
        "w2": np.ascontiguousarray(w2, np.float32),
        "gamma1": np.ascontiguousarray(gamma1, np.float32),
        "beta1": np.ascontiguousarray(beta1, np.float32),
        "gamma2": np.ascontiguousarray(gamma2, np.float32),
        "beta2": np.ascontiguousarray(beta2, np.float32),
    }
    in_maps = [{"x": shard[i], **common} for i in range(N_CORES)]
    old_m = nc.m
    nc.m = get_hw_module(nc.m)
    try:
        res = run_bass_kernel_spmd(nc, in_maps, core_ids=list(range(N_CORES)))
    finally:
        nc.m = old_m
    out = np.concatenate([res.results[i]["out"] for i in range(N_CORES)], axis=0)
    return out.astype(np.float32)


# revision 38
# speedup vs baseline: 1.0290x; 1.0290x over previous
"""Trainium2 Bass kernel for a DoReFa-quantized ResNet BasicBlock.

    out = act(bn2(conv3x3(act(bn1(conv3x3(x, qw(w1)))), qw(w2))) + x)

with 4-bit DoReFa weight/activation quantization and training-mode BatchNorm
(batch statistics over N,H,W).

Strategy (8 NeuronCores, data-parallel over batch):
 - batch N=64 sharded 8 images/core; weights replicated.
 - BN uses *synced* batch statistics: per-core per-channel mean/var come from
   one-pass vector bn_stats/bn_aggr, converted to sum/sumsq and AllGathered
   across the 8 cores (two tiny [128,2] collectives).
 - conv3x3 = 9 shifted matmuls accumulated in PSUM (C_in on partitions,
   pixels on free dim), using a zero-padded [C,58,58] SBUF image.
 - Weight quantization produces small odd integers (2m-15, |.| <= 15) which
   are exact in bf16/fp8; the /15 scales are folded into the BN affine maps.
   conv1 runs single-pass bf16 (x rounded once to bf16, ~2^-9 relative);
   conv2 inputs are exact small ints and runs in fp8 exactly, with DoubleRow
   perf mode fusing tap pairs (two 3x3 taps per matmul).
 - Activation quantization uses the 2^23 magic-constant round-to-nearest-even
   (matches jnp.round) after clipping via min/max; the final residual path is
   computed on the x15 scale so relu folds into the rounding pass.
"""

import numpy as np

import bass_rust
import concourse.bacc as bacc
import concourse.mybir as mybir
import concourse.tile as tile
import concourse.bass_isa as bass_isa
from concourse.bass_utils import run_bass_kernel_spmd
from concourse.bass_interp import get_hw_module
from concourse.masks import make_identity

F32 = mybir.dt.float32
BF16 = mybir.dt.bfloat16
F16 = mybir.dt.float16
FP8 = mybir.dt.float8e4
AF = mybir.ActivationFunctionType
ALU = mybir.AluOpType
DR = mybir.MatmulPerfMode.DoubleRow

N_CORES = 8
N_PER = 8            # images per core
C = 128              # channels
H = W = 56
HW = H * W           # 3136
QHW = HW // 4        # quarter image pixels
PW = 58              # padded height/width
RCH = 8              # output rows per chunk
NCHUNK = H // RCH    # 7 chunks per image
CHN = RCH * W        # 448 pixels per chunk
NREC = N_PER * NCHUNK
MAGIC = float(2.0 ** 23)
N_SHARD = float(N_PER * HW)           # per-core BN sample count
N_BATCH = 64 * HW                     # full-batch BN sample count
INV_N = float(np.float32(1.0 / N_BATCH))
EPS1 = float(np.float32(225e-5))      # 15^2 * 1e-5   (conv1 output scale)
EPS2 = float(np.float32(50625e-5))    # 225^2 * 1e-5  (conv2 output scale)
INV15 = float(np.float32(1.0 / 15.0))
# tanh(w) ~ w * (1 + w2*(c1 + w2*(c2 + w2*c3))), |w| < 0.25
TC1 = float(np.float32(-1.0 / 3.0))
TC2 = float(np.float32(2.0 / 15.0))
TC3 = float(np.float32(-17.0 / 315.0))
TAPS = [(ky, kx) for ky in range(3) for kx in range(3)]
TAP_OFF = [ky * PW + kx for ky, kx in TAPS]

_CACHED = {}


def _pair_rhs(apad_ap, r0, p):
    """Overlapping AP selecting the two shifted conv windows of tap pair p:
    [C, 2, RCH, W] where dim1 steps between tap offsets (DoubleRow rhs)."""
    ky0, kx0 = TAPS[2 * p]
    delta = TAP_OFF[2 * p + 1] - TAP_OFF[2 * p]
    base = apad_ap[:, r0 + ky0:r0 + ky0 + RCH, kx0:kx0 + W]
    u = base.unsqueeze(1).broadcast_to((C, 2, RCH, W)).copy()
    pairs = [tuple(x) for x in u.ap]
    pairs[1] = (delta, 2)
    u.ap = bass_rust.VecI64Pair(pairs)
    return u


def _border_zero(nc, pad_ap):
    """Zero just the 1-px border ring of a [C, PW, PW] padded tile (the
    interior is fully overwritten) - 4 small DVE memsets instead of one
    slow full-tile gpsimd memset."""
    nc.vector.memset(pad_ap[:, 0:1, :], 0.0)
    nc.vector.memset(pad_ap[:, PW - 1:PW, :], 0.0)
    nc.vector.memset(pad_ap[:, 1:PW - 1, 0:1], 0.0)
    nc.vector.memset(pad_ap[:, 1:PW - 1, PW - 1:PW], 0.0)


def _tanh_poly_multi(nc, parts):
    """wt = taylor_tanh(w) elementwise over several (out, tmp, w2, w) slice
    groups, ops interleaved across groups to hide DVE inter-op latency."""
    steps = [
        lambda o, t, w2, w: nc.vector.tensor_tensor(w2, w, w, ALU.mult),
        lambda o, t, w2, w: nc.vector.tensor_scalar(t, w2, TC3, TC2,
                                                    ALU.mult, ALU.add),
        lambda o, t, w2, w: nc.vector.tensor_tensor(t, t, w2, ALU.mult),
        lambda o, t, w2, w: nc.vector.tensor_scalar(t, t, TC1, None, ALU.add),
        lambda o, t, w2, w: nc.vector.tensor_tensor(t, t, w2, ALU.mult),
        lambda o, t, w2, w: nc.vector.tensor_tensor(o, w, t, ALU.mult),
        lambda o, t, w2, w: nc.vector.tensor_tensor(o, w, o, ALU.add),
    ]
    for step in steps:
        for grp in parts:
            step(*grp)


def _tanh_poly(nc, tt_out, ts_tmp, w2src, wsrc):
    _tanh_poly_multi(nc, [(tt_out, ts_tmp, w2src, wsrc)])


def _quant_stats(nc, consts, wsb, name):
    """Global max |w| across all partitions (the only gpsimd step of the
    weight quant - hoisted early so it is not stuck behind the warm-up
    collective trigger, which blocks the gpsimd queue until the NEFF init
    barrier completes)."""
    amax = consts.tile([C, 1], F32, tag=f"amax{name}")
    nc.vector.tensor_reduce(amax[:], wsb, mybir.AxisListType.X, ALU.max,
                            apply_absolute_value=True)
    gmax = consts.tile([C, 1], F32, tag=f"gmax{name}")
    nc.gpsimd.partition_all_reduce(gmax[:], amax[:], C, bass_isa.ReduceOp.max)
    return gmax


def _quant_chain(nc, wqp, consts, wsb, gmax, name):
    """DoReFa-quantize one [128,128,3,3] weight (already DMA'd into wsb
    [C, C*9]): pure DVE chain producing the bf16 integer-code tile wi."""
    K = C * 9
    # tanh(max) (tanh is monotone; same f32 poly as below)
    mt1 = consts.tile([C, 1], F32, tag=f"mt1{name}")
    mt2 = consts.tile([C, 1], F32, tag=f"mt2{name}")
    mval = consts.tile([C, 1], F32, tag=f"mval{name}")
    _tanh_poly(nc, mval[:], mt1[:], mt2[:], gmax[:])
    # s15 = 15 / (2*M); wn15 = wt*s15 + 7.5; codes = rtne(wn15)
    inv2m = consts.tile([C, 1], F32, tag=f"inv2m{name}")
    nc.vector.tensor_scalar(inv2m[:], mval[:], 2.0, None, ALU.mult)
    nc.vector.reciprocal(inv2m[:], inv2m[:])
    s15 = consts.tile([C, 1], F32, tag=f"s15{name}")
    nc.vector.tensor_scalar(s15[:], inv2m[:], 15.0, None, ALU.mult)
    w2t = wqp.tile([C, K], F32, tag="w2t")
    qt = wqp.tile([C, K], F32, tag="qt")
    wt = wqp.tile([C, K], F32, tag="wt")
    wn = wqp.tile([C, K], F32, tag="wn")
    wi = wqp.tile([C, K], BF16, tag="wi")
    HC = K // 2
    halves = [slice(0, HC), slice(HC, K)]
    # two interleaved half-column chains hide DVE inter-op latency
    _tanh_poly_multi(nc, [(wt[:, hs], qt[:, hs], w2t[:, hs], wsb[:, hs])
                          for hs in halves])
    for hs in halves:
        nc.vector.tensor_scalar(wn[:, hs], wt[:, hs], s15[:, 0:1], 7.5,
                                ALU.mult, ALU.add)
    for hs in halves:
        nc.vector.tensor_scalar(wn[:, hs], wn[:, hs], MAGIC, -MAGIC,
                                ALU.add, ALU.add)
    for hs in halves:
        nc.vector.tensor_scalar(wi[:, hs], wn[:, hs], 2.0, -15.0,
                                ALU.mult, ALU.add)
    return wi


def _quant_transposes(nc, ptr, ident, wi, copy_fn):
    """Transpose each tap via PE: lhsT[i, o] = Wi[o, i*9+t]."""
    wir = wi.rearrange("o (i t) -> o i t", t=9)
    for t in range(9):
        pst = ptr.tile([C, C], BF16, tag="tr")
        nc.tensor.transpose(pst[:], wir[:, :, t], ident[:])
        copy_fn(t, pst)


def _stats_to_sums(nc, statsp, stats_rec, nsamp, name):
    """bn_aggr [C, k, 6] records into [C,2] (mean,var), convert to
    [sum, sumsq] over the nsamp samples they cover."""
    mv = statsp.tile([C, 2], F32, tag=f"mv{name}")
    nc.vector.bn_aggr(mv[:], stats_rec)
    msq = statsp.tile([C, 1], F32, tag=f"msq{name}")
    nc.vector.tensor_tensor(msq[:], mv[:, 0:1], mv[:, 0:1], ALU.mult)
    st = statsp.tile([C, 2], F32, tag=f"st{name}")
    nc.vector.tensor_scalar(st[:, 0:1], mv[:, 0:1], nsamp, None, ALU.mult)
    nc.vector.tensor_tensor(msq[:], mv[:, 1:2], msq[:], ALU.add)
    nc.vector.tensor_scalar(st[:, 1:2], msq[:], nsamp, None, ALU.mult)
    return st


def _ag_sum(nc, statsp, dram, st, RG, name):
    """Cross-core sum of a [C,2] stats tile via AllGather + local reduce
    (AG has a lower latency floor than AllReduce for tiny payloads).
    The staging DMAs ride the sync queue - its ring is kept warm by the
    image loads/prefetches, while a cold ring adds ~6us latency."""
    agi = dram.tile([C, 2], F32, tag=f"agi{name}")
    ago = dram.tile([N_CORES, C, 2], F32, tag=f"ago{name}")
    nc.sync.dma_start(agi[:], st[:])
    nc.gpsimd.collective_compute(
        "AllGather", ALU.bypass, replica_groups=RG,
        ins=[agi.opt()], outs=[ago.opt()])
    allst = statsp.tile([C, 2, N_CORES], F32, tag=f"allst{name}")
    nc.sync.dma_start(allst[:], ago.rearrange("r c s -> c s r"))
    rst = statsp.tile([C, 2], F32, tag=f"rst{name}")
    nc.vector.tensor_reduce(rst[:], allst[:], mybir.AxisListType.X, ALU.add)
    return rst


def _load_gb(nc, consts, gamma_ap, beta_ap, name):
    """Preload gamma/beta on the scalar DMA queue (done at build start so
    they are never stuck behind the phase-3 x prefetch on the sync queue)."""
    g = consts.tile([C, 1], F32, tag=f"g{name}")
    nc.scalar.dma_start(g[:], gamma_ap.rearrange("(c one) -> c one", one=1))
    b = consts.tile([C, 1], F32, tag=f"b{name}")
    nc.scalar.dma_start(b[:], beta_ap.rearrange("(c one) -> c one", one=1))
    return g, b


def _bn_vectors(nc, consts, rstats, g, b, eps, post_scale, name):
    """Build per-channel scale/bias [128,1] s.t. T*scale + bias equals
    post_scale * batchnorm(T/k); eps is pre-scaled by k^2."""
    mean = consts.tile([C, 1], F32, tag=f"mean{name}")
    nc.vector.tensor_scalar(mean[:], rstats[:, 0:1], INV_N, None, ALU.mult)
    var = consts.tile([C, 1], F32, tag=f"var{name}")
    nc.vector.tensor_scalar(var[:], rstats[:, 1:2], INV_N, None, ALU.mult)
    msq = consts.tile([C, 1], F32, tag=f"msq{name}")
    nc.vector.tensor_tensor(msq[:], mean[:], mean[:], ALU.mult)
    nc.vector.tensor_tensor(var[:], var[:], msq[:], ALU.subtract)
    epst = consts.tile([C, 1], F32, tag=f"eps{name}")
    nc.vector.memset(epst[:], eps)
    std = consts.tile([C, 1], F32, tag=f"std{name}")
    nc.scalar.activation(std[:], var[:], AF.Sqrt, bias=epst[:, 0:1], scale=1.0)
    inv = consts.tile([C, 1], F32, tag=f"inv{name}")
    nc.vector.reciprocal(inv[:], std[:])
    scale = consts.tile([C, 1], F32, tag=f"scale{name}")
    nc.vector.tensor_tensor(scale[:], g[:], inv[:], ALU.mult)
    nc.vector.tensor_scalar(scale[:], scale[:], post_scale, None, ALU.mult)
    bias = consts.tile([C, 1], F32, tag=f"bias{name}")
    nc.vector.tensor_tensor(bias[:], mean[:], scale[:], ALU.mult)
    nc.vector.tensor_scalar(b[:], b[:], post_scale, None, ALU.mult)
    nc.vector.tensor_tensor(bias[:], b[:], bias[:], ALU.subtract)
    return scale, bias


def build():
    nc = bacc.Bacc("TRN2", target_bir_lowering=False, debug=False,
                   num_devices=N_CORES)
    x_ap = nc.dram_tensor("x", [N_PER, C, H, W], F32, kind="ExternalInput").ap()
    w1_ap = nc.dram_tensor("w1", [C, C, 3, 3], F32, kind="ExternalInput").ap()
    w2_ap = nc.dram_tensor("w2", [C, C, 3, 3], F32, kind="ExternalInput").ap()
    g1_ap = nc.dram_tensor("gamma1", [C], F32, kind="ExternalInput").ap()
    b1_ap = nc.dram_tensor("beta1", [C], F32, kind="ExternalInput").ap()
    g2_ap = nc.dram_tensor("gamma2", [C], F32, kind="ExternalInput").ap()
    b2_ap = nc.dram_tensor("beta2", [C], F32, kind="ExternalInput").ap()
    out_ap = nc.dram_tensor("out", [N_PER, C, H, W], F32,
                            kind="ExternalOutput").ap()
    x_r = x_ap.rearrange("n c h w -> n c h w")
    x_f = x_ap.rearrange("n c h w -> n c (h w)")
    out_f = out_ap.rearrange("n c h w -> n c (h w)")
    RG = [list(range(N_CORES))]

    with tile.TileContext(nc) as tc:
        with tc.tile_pool(name="consts", bufs=1) as consts, \
             tc.tile_pool(name="T", bufs=N_PER) as pool_T, \
             tc.tile_pool(name="wq", bufs=1) as wqp, \
             tc.tile_pool(name="padhl", bufs=3) as padhl, \
             tc.tile_pool(name="apad", bufs=3) as apadp, \
             tc.tile_pool(name="xio", bufs=2) as xio, \
             tc.tile_pool(name="ximg", bufs=6) as ximgp, \
             tc.tile_pool(name="yimg", bufs=5) as yimgp, \
             tc.tile_pool(name="psum", bufs=5, space="PSUM") as psum, \
             tc.tile_pool(name="ptr", bufs=2, space="PSUM") as ptr, \
             tc.tile_pool(name="stats", bufs=1) as statsp, \
             tc.tile_pool(name="dram", bufs=1, space="DRAM") as dram:

            # weight DMAs issue first on the scalar queue so the wquant
            # chain is never stuck behind the 6.4MB of image loads
            wsb = wqp.tile([C, 2 * C * 9], F32, tag="wsb")
            nc.scalar.dma_start(wsb[:, 0:C * 9],
                                w1_ap.rearrange("o i kh kw -> o (i kh kw)"))
            nc.scalar.dma_start(wsb[:, C * 9:],
                                w2_ap.rearrange("o i kh kw -> o (i kh kw)"))

            ident = consts.tile([C, C], BF16, tag="ident")
            make_identity(nc, ident[:])
            lhsT1 = consts.tile([C, 9, C], F16, tag="lhsT1")
            # conv2 weights: 4 DoubleRow pairs + 1 single, fp8
            lhsT2p = consts.tile([C, 4, 2, C], FP8, tag="lhsT2p")
            lhsT2s = consts.tile([C, C], FP8, tag="lhsT2s")

            # lhsT copies ride the DVE queue: the ACT queue carries the
            # image-prep copies, which wait on image DMAs - Tile's
            # counter-based sems would make conv1's first matmul wait for
            # ALL earlier ACT work (tens of us of DMA) otherwise.
            def copy1(t, pst):
                nc.vector.tensor_copy(lhsT1[:, t, :], pst[:])

            def copy2(t, pst):
                if t < 8:
                    nc.vector.tensor_copy(lhsT2p[:, t // 2, t % 2, :], pst[:])
                else:
                    nc.vector.tensor_copy(lhsT2s[:], pst[:])

            def prep_image(i):
                xp = padhl.tile([C, PW, PW], F16, tag="pad")
                _border_zero(nc, xp)
                for g in range(4):
                    xs = xio.tile([C, 14, W], F32, tag="xio")
                    nc.sync.dma_start(xs[:], x_r[i, :, g * 14:(g + 1) * 14, :])
                    nc.scalar.copy(xp[:, 1 + g * 14:1 + (g + 1) * 14, 1:57],
                                   xs[:])
                return xp

            g1, b1 = _load_gb(nc, consts, g1_ap, b1_ap, "1")
            g2, b2 = _load_gb(nc, consts, g2_ap, b2_ap, "2")
            prefetched = []

            stats1 = statsp.tile([C, NREC, 6], F32, tag="stats1")
            T1 = []

            def conv1_image(i, xp):
                Ti = pool_T.tile([C, HW], F32, tag="T")
                T1.append(Ti)
                Tir = Ti.rearrange("c (h w) -> c h w", w=W)
                for ck in range(NCHUNK):
                    ps = psum.tile([C, CHN], F32, tag="mm")
                    r0 = ck * RCH
                    for k, (ky, kx) in enumerate(TAPS):
                        rhs = xp[:, r0 + ky:r0 + ky + RCH, kx:kx + W]
                        nc.tensor.matmul(ps[:], lhsT1[:, k, :], rhs,
                                         start=(k == 0), stop=(k == 8))
                    col = i * NCHUNK + ck
                    nc.scalar.copy(Tir[:, r0:r0 + RCH, :], ps[:])
                    nc.vector.bn_stats(stats1[:, col, :], ps[:])
                if i == 6:
                    # mid-flight cross-core sync to absorb skew before AR1
                    ccs = dram.tile([C, 6], F32, tag="ccs")
                    ccso = dram.tile([C, 6], F32, tag="ccso")
                    nc.sync.dma_start(ccs[:], stats1[:, 6 * NCHUNK, :])
                    nc.gpsimd.collective_compute(
                        "AllReduce", ALU.add, replica_groups=RG,
                        ins=[ccs.opt()], outs=[ccso.opt()])

            # prep image 0 first (only DMA/ACT/DVE), then quantize w1 so
            # conv1 can start as soon as lhsT1 is ready.  The warm-up
            # collective trigger goes AFTER wquant1's gpsimd reduce (the
            # trigger blocks the gpsimd queue until the cc stream is up),
            # and wquant2 goes AFTER conv1 image 0 (its PE transposes would
            # otherwise delay conv1's matmuls in the in-order tensor queue).
            xp0 = prep_image(0)
            gmax1 = _quant_stats(nc, consts, wsb[:, 0:C * 9], "1")
            gmax2 = _quant_stats(nc, consts, wsb[:, C * 9:], "2")

            # dummy collective to absorb mesh/barrier warmup during conv1;
            # its input is copied from gmax2 to force the gpsimd queue to
            # run both partition reduces BEFORE the trigger (the trigger
            # stalls the queue until the NEFF init barrier completes)
            ccwi = dram.tile([C, 1], F32, tag="ccwi")
            ccwo = dram.tile([C, 1], F32, tag="ccwo")
            nc.gpsimd.dma_start(ccwi[:], gmax2[:])
            nc.gpsimd.collective_compute(
                "AllReduce", ALU.add, replica_groups=RG,
                ins=[ccwi.opt()], outs=[ccwo.opt()])

            # w1's (short) DVE chain gates conv1's start; w2's chain runs
            # in DVE slack during images 0-3 and its PE transposes enter
            # the in-order tensor queue only after image 3's matmuls.
            with nc.named_scope("wquant1"):
                wi1 = _quant_chain(nc, wqp, consts, wsb[:, 0:C * 9],
                                   gmax1, "1")
                _quant_transposes(nc, ptr, ident, wi1, copy1)
            xp1 = prep_image(1)
            with nc.named_scope("conv1_img0"):
                conv1_image(0, xp0)
            with nc.named_scope("wquant2_chain"):
                wi2 = _quant_chain(nc, wqp, consts, wsb[:, C * 9:],
                                   gmax2, "2")

            # ---------------- phase 1: conv1 + stats ----------------
            st1a = None
            with nc.named_scope("conv1"):
                for i in range(1, N_PER):
                    xp = xp1 if i == 1 else prep_image(i)
                    conv1_image(i, xp)
                    if i == 3:
                        with nc.named_scope("wquant2_tr"):
                            _quant_transposes(nc, ptr, ident, wi2, copy2)
                    if i == 6:
                        # aggregate images 0-6 while image 7 is convolving
                        st1a = _stats_to_sums(nc, statsp,
                                              stats1[:, 0:7 * NCHUNK, :],
                                              float(7 * HW), "1a")

            # allreduce stats 1 (image 7's records + the precomputed rest)
            st1b = _stats_to_sums(nc, statsp, stats1[:, 7 * NCHUNK:, :],
                                  float(HW), "1b")
            st1 = statsp.tile([C, 2], F32, tag="st1")
            nc.vector.tensor_tensor(st1[:], st1a[:], st1b[:], ALU.add)
            rst1 = _ag_sum(nc, statsp, dram, st1, RG, "1")
            sc1, bi1 = _bn_vectors(nc, consts, rst1, g1, b1, EPS1, 15.0, "1")

            # ---------------- phase 2: act1 + conv2 + stats ----------------
            stats2 = statsp.tile([C, NREC, 6], F32, tag="stats2")
            T2 = []
            with nc.named_scope("act1_conv2"):
                for i in range(N_PER):
                    ap_t = apadp.tile([C, PW, PW], FP8, tag="apad")
                    _border_zero(nc, ap_t)
                    Tir = T1[i]
                    for q in range(4):
                        y = yimgp.tile([C, 14, W], F32, tag="yimg")
                        nc.scalar.activation(
                            y[:], Tir[:, q * QHW:(q + 1) * QHW].rearrange(
                                "c (h w) -> c h w", w=W),
                            AF.Relu, bias=bi1[:, 0:1], scale=sc1[:, 0:1])
                        nc.vector.tensor_scalar(y[:], y[:], 15.0, MAGIC,
                                                ALU.min, ALU.add)
                        nc.scalar.activation(
                            ap_t[:, 1 + q * 14:1 + (q + 1) * 14, 1:57], y[:],
                            AF.Copy, bias=-MAGIC, scale=1.0)
                    Ti2 = pool_T.tile([C, HW], F32, tag="T")
                    T2.append(Ti2)
                    T2r = Ti2.rearrange("c (h w) -> c h w", w=W)
                    for ck in range(NCHUNK):
                        ps = psum.tile([C, CHN], F32, tag="mm")
                        r0 = ck * RCH
                        for p in range(4):
                            nc.tensor.matmul(ps[:], lhsT2p[:, p, :, :],
                                             _pair_rhs(ap_t, r0, p),
                                             start=(p == 0), stop=False,
                                             perf_mode=DR)
                        rhs8 = ap_t[:, r0 + 2:r0 + 2 + RCH, 2:2 + W]
                        nc.tensor.matmul(ps[:], lhsT2s[:], rhs8,
                                         start=False, stop=True)
                        col = i * NCHUNK + ck
                        nc.any.tensor_copy(T2r[:, r0:r0 + RCH, :], ps[:])
                        nc.vector.bn_stats(stats2[:, col, :], ps[:])
                    # prefetch the residual x quarters for phase 3 on the
                    # (otherwise idle) sync DMA queue - only as many as the
                    # pool holds, so the stats-AG staging DMAs behind them
                    # on the ring are never gated on phase-3 progress
                    while len(prefetched) < 6:
                        j = len(prefetched)
                        pi, pq = divmod(j, 4)
                        xr = ximgp.tile([C, QHW], F32, tag="ximg")
                        nc.sync.dma_start(
                            xr[:], x_f[pi, :, pq * QHW:(pq + 1) * QHW])
                        prefetched.append(xr)
                    if i == 6:
                        # mid-flight cross-core sync to absorb skew
                        # before AG2 (cc stream executes in order, so AG2
                        # starts skew-free right after this completes)
                        ccs2 = dram.tile([C, 6], F32, tag="ccs2")
                        ccso2 = dram.tile([C, 6], F32, tag="ccso2")
                        nc.sync.dma_start(ccs2[:], stats2[:, 6 * NCHUNK, :])
                        nc.gpsimd.collective_compute(
                            "AllReduce", ALU.add, replica_groups=RG,
                            ins=[ccs2.opt()], outs=[ccso2.opt()])
                        st2a = _stats_to_sums(nc, statsp,
                                              stats2[:, 0:7 * NCHUNK, :],
                                              float(7 * HW), "2a")

            # allreduce stats 2 (image 7's records + the precomputed rest)
            st2b = _stats_to_sums(nc, statsp, stats2[:, 7 * NCHUNK:, :],
                                  float(HW), "2b")
            st2 = statsp.tile([C, 2], F32, tag="st2")
            nc.vector.tensor_tensor(st2[:], st2a[:], st2b[:], ALU.add)
            rst2 = _ag_sum(nc, statsp, dram, st2, RG, "2")
            # fold the x15 residual scale into the bn2 affine: y15 = 15*bn2
            sc2, bi2 = _bn_vectors(nc, consts, rst2, g2, b2, EPS2, 15.0, "2")

            # ------------- phase 3: bn2 + residual + act -> out -------------
            # v15 = 15*bn2(T2) + 15*x; codes = relu(rtne(min(v15,15)));
            # out = codes/15
            with nc.named_scope("final"):
                for i in range(N_PER):
                    for q in range(4):
                        sl = slice(q * QHW, (q + 1) * QHW)
                        xr = prefetched[i * 4 + q]
                        # keep the rolling x prefetch ~10 quarters ahead
                        if len(prefetched) < 4 * N_PER:
                            j = len(prefetched)
                            pi, pq = divmod(j, 4)
                            nxr = ximgp.tile([C, QHW], F32, tag="ximg")
                            nc.sync.dma_start(
                                nxr[:], x_f[pi, :, pq * QHW:(pq + 1) * QHW])
                            prefetched.append(nxr)
                        y = yimgp.tile([C, QHW], F32, tag="yimg")
                        nc.scalar.activation(
                            y[:], T2[i][:, sl],
                            AF.Identity, bias=bi2[:, 0:1], scale=sc2[:, 0:1])
                        nc.vector.scalar_tensor_tensor(
                            y[:], xr[:], 15.0, y[:], ALU.mult, ALU.add)
                        nc.vector.tensor_scalar(y[:], y[:], 15.0, MAGIC,
                                                ALU.min, ALU.add)
                        nc.vector.tensor_scalar(y[:], y[:], -MAGIC, 0.0,
                                                ALU.add, ALU.max)
                        # write back into the (now dead) T2 quarter so the
                        # image stores as one contiguous [C,3136] DMA
                        nc.scalar.activation(T2[i][:, sl], y[:], AF.Copy,
                                             bias=0.0, scale=INV15)
                    nc.gpsimd.dma_start(out_f[i, :, :], T2[i][:])

    nc.compile()
    return nc


def kernel(x, w1, w2, gamma1, beta1, gamma2, beta2):
    if "nc" not in _CACHED:
        _CACHED["nc"] = build()
    nc = _CACHED["nc"]
    x = np.ascontiguousarray(x, dtype=np.float32)
    shard = x.reshape(N_CORES, N_PER, C, H, W)
    common = {
        "w1": np.ascontiguousarray(w1, np.float32),
        "w2": np.ascontiguousarray(w2, np.float32),
        "gamma1": np.ascontiguousarray(gamma1, np.float32),
        "beta1": np.ascontiguousarray(beta1, np.float32),
        "gamma2": np.ascontiguousarray(gamma2, np.float32),
        "beta2": np.ascontiguousarray(beta2, np.float32),
    }
    in_maps = [{"x": shard[i], **common} for i in range(N_CORES)]
    old_m = nc.m
    nc.m = get_hw_module(nc.m)
    try:
        res = run_bass_kernel_spmd(nc, in_maps, core_ids=list(range(N_CORES)))
    finally:
        nc.m = old_m
    out = np.concatenate([res.results[i]["out"] for i in range(N_CORES)], axis=0)
    return out.astype(np.float32)
